# revision 5
# baseline (speedup 1.0000x reference)
"""EnhancedCorrelationGNN Trainium2 kernel (8 NeuronCores, SPMD).

Strategy: destination-sorted edge processing with node-range output sharding,
fully collective-free.
 - Host (free): counting-sort edges by dst, partition nodes into 8 ranges of
   6272 (49 blocks x 128 nodes per core). Per core the node table is ROTATED
   so its own slice comes first; per block, edges are split by rotated src
   half (dma_gather int16 index limit) and padded to 128-edge tiles with
   cross-core-uniform tile counts (one SPMD program).
 - Phase 1 (device): EVERY core computes the FULL node table from the
   replicated x input: h = x @ W plus both attention projections in ONE bf16
   matmul per 128-node tile (rhs = [W@a_dst | W | W@a_src] prepped on host),
   writes bf16 [h|as] rows (512B) to local DRAM. attn_d for the core's own
   49 blocks stays in SBUF. No AllGather.
 - Phase 2 (device): per 32-tile chunk, one dma_gather of bf16 [h|as] rows
   by src (512B/edge); attn_d is expanded per-edge by a TensorE matmul with
   a host-shipped fp8 one-hot (node x edge) instead of a second gather.
   Scores: DVE adds + ACT Lrelu/Exp; messages bf16; one-hot segment matrix
   via is_equal(dstl, iota) in bf16; per-tile bf16 TensorE matmul
   scatter-accumulates [msgs | p] into the block PSUM; per block normalize
   by 1/(sum p + 1e-10), add bias, DMA out.
"""
import sys

if "/opt/trn_rl_repo" not in sys.path:
    sys.path.insert(0, "/opt/trn_rl_repo")

import numpy as np
import ml_dtypes

import concourse.bass as bass
import concourse.bacc as bacc
import concourse.mybir as mybir
import concourse.tile as tile
from concourse.bass_utils import run_bass_kernel_spmd

# ---------------------------------------------------------------- constants
N = 50000
E = 800000
IN_F = 128
H = 8
HD = 16
OUT_F = H * HD          # 128
ALPHA = 0.2
EPS = 1e-10

NCORES = 8
P = 128
NPC = 6272              # nodes per core = 49 * 128; 8*6272 = 50176 >= N
NPAD = NCORES * NPC     # 50176
NBLK = NPC // P         # 49
HALF = NPAD // 2        # 25088 rotated-table rows per gather stream

ROW = 256               # table row elems (bf16): h(128) | as(8) | pad -> 512B
AS_OFF = 128            # attn_s offset within row
CHUNK_TILES = 32        # tiles per gather/DVE chunk
IDX_COLS = CHUNK_TILES * P // 16   # wrapped int16 idx columns per chunk
PAD_DSTL = 300.0        # one-hot miss sentinel (exact in bf16)
XBLK = 28               # phase-1 blocks per xT chunk; 392 = 14 * 28
NXCH = (NPAD // P) // XBLK         # 14 phase-1 chunks (7 per half)

FP = mybir.dt.float32
BF = mybir.dt.bfloat16
F8 = mybir.dt.float8e4
NP_BF = ml_dtypes.bfloat16
NP_F8 = ml_dtypes.float8_e4m3

USE_FP8_S2 = True       # one-hot S2 dtype (fp8 halves its DMA vs bf16)


# ---------------------------------------------------------------- planning
def _cdiv(a, b):
    return -(-a // b)


def _wrap_idx(idx_flat: np.ndarray) -> np.ndarray:
    """[n] -> [128, IDX_COLS] int16: idx j at [j%16, j//16], replicated x8."""
    n = idx_flat.shape[0]
    assert n % 16 == 0
    w = idx_flat.reshape(n // 16, 16).T.astype(np.int16)      # [16, n/16]
    w = np.tile(w, (8, 1))                                    # [128, n/16]
    out = np.zeros((P, IDX_COLS), dtype=np.int16)
    out[:, : w.shape[1]] = w
    return out


def plan_and_inputs(edge_index, edge_weight):
    """Host-side edge partitioning. Returns (plan, per_core_arrays).

    plan (core-independent, defines the SPMD program):
      KA, KB: [NBLK] tiles per (block, half)
      chunks: list of dicts(stream, g0, nt) over stream-major tile ids
      block_tiles: per block, list of (chunk_id, slot) in matmul order
      tile_block: [T] block id of each global tile
      T, n_chunks
    per_core_arrays[c]:
      src_idx [n_chunks,128,IDX_COLS] i16 (stream-relative, rotated table)
      dstl    [128, T] bf16 (block-relative dst, PAD_DSTL for pad slots)
      ew      [128, T] bf16
      s2      [128, T*128] fp8/bf16 one-hot: s2[n, t*128+e] = (dstl[e,t]==n)
    """
    src = np.asarray(edge_index[0], dtype=np.int64)
    dst = np.asarray(edge_index[1], dtype=np.int64)
    ew = np.asarray(edge_weight, dtype=np.float32)

    order = np.argsort(dst, kind="stable")
    src_s, dst_s, ew_s = src[order], dst[order], ew[order]

    # block boundaries over sorted dst
    blk_starts = np.searchsorted(dst_s, np.arange(0, NPAD + 1, P))
    # per (core, block, half) edge index lists (into the sorted arrays)
    cnt = np.zeros((NCORES, NBLK, 2), dtype=np.int64)
    lists = [[[None, None] for _ in range(NBLK)] for _ in range(NCORES)]
    rot_all = []
    for c in range(NCORES):
        rot = (src_s - c * NPC) % NPAD     # rotated src row per core
        rot_all.append(rot)
        for b in range(NBLK):
            g = c * NBLK + b
            lo, hi = blk_starts[g], blk_starts[g + 1]
            r = rot[lo:hi]
            mA = r < HALF
            idxs = np.arange(lo, hi)
            lists[c][b][0] = idxs[mA]
            lists[c][b][1] = idxs[~mA]
            cnt[c, b, 0] = mA.sum()
            cnt[c, b, 1] = (~mA).sum()

    KA = np.maximum(_cdiv(cnt[:, :, 0].max(axis=0), P), 1).astype(np.int64)
    KB = _cdiv(cnt[:, :, 1].max(axis=0), P).astype(np.int64)

    T_A = int(KA.sum())
    T_B = int(KB.sum())
    T = T_A + T_B
    cumKA = np.concatenate([[0], np.cumsum(KA)])
    cumKB = np.concatenate([[0], np.cumsum(KB)])

    # chunks: stream-major [0,T_A) then [T_A,T)
    chunks = []
    g = 0
    while g < T_A:
        nt = min(CHUNK_TILES, T_A - g)
        chunks.append(dict(stream=0, g0=g, nt=nt))
        g += nt
    while g < T:
        nt = min(CHUNK_TILES, T - g)
        chunks.append(dict(stream=1, g0=g, nt=nt))
        g += nt
    n_chunks = len(chunks)

    chunk_of = np.empty(T, dtype=np.int64)
    slot_of = np.empty(T, dtype=np.int64)
    for ci, ch in enumerate(chunks):
        chunk_of[ch["g0"]: ch["g0"] + ch["nt"]] = ci
        slot_of[ch["g0"]: ch["g0"] + ch["nt"]] = np.arange(ch["nt"])

    tile_block = np.empty(T, dtype=np.int64)
    block_tiles = []
    for b in range(NBLK):
        tl = []
        for k in range(KA[b]):
            gidx = cumKA[b] + k
            tile_block[gidx] = b
            tl.append((int(chunk_of[gidx]), int(slot_of[gidx])))
        for k in range(KB[b]):
            gidx = T_A + cumKB[b] + k
            tile_block[gidx] = b
            tl.append((int(chunk_of[gidx]), int(slot_of[gidx])))
        block_tiles.append(tl)

    plan = dict(KA=KA, KB=KB, T=T, T_A=T_A, chunks=chunks,
                block_tiles=block_tiles, tile_block=tile_block,
                n_chunks=n_chunks)

    # ---------------- per-core slot arrays
    s2_dt = NP_F8 if USE_FP8_S2 else NP_BF
    per_core = []
    for c in range(NCORES):
        src_rel = np.zeros((T, P), dtype=np.int16)
        dstl = np.full((T, P), PAD_DSTL, dtype=np.float32)
        eww = np.zeros((T, P), dtype=np.float32)
        rot = rot_all[c]
        for b in range(NBLK):
            for half, K, cum, base in ((0, KA, cumKA, 0),
                                       (1, KB, cumKB, T_A)):
                idxs = lists[c][b][half]
                # src-sorted slots: consecutive gather rows are nearly
                # contiguous in DRAM (page locality)
                idxs = idxs[np.argsort(rot[idxs], kind="stable")]
                n = idxs.shape[0]
                g0 = base + cum[b]
                nslots = int(K[b]) * P
                s_loc = np.zeros(nslots, dtype=np.int64)
                dl = np.full(nslots, PAD_DSTL, dtype=np.float32)
                w = np.zeros(nslots, dtype=np.float32)
                if n:
                    s_loc[:n] = rot[idxs] - (HALF if half else 0)
                    dl[:n] = (dst_s[idxs] - (c * NPC + b * P)).astype(
                        np.float32)
                    w[:n] = ew_s[idxs]
                src_rel[g0: g0 + int(K[b])] = s_loc.reshape(
                    int(K[b]), P).astype(np.int16)
                dstl[g0: g0 + int(K[b])] = dl.reshape(int(K[b]), P)
                eww[g0: g0 + int(K[b])] = w.reshape(int(K[b]), P)

        src_idx = np.zeros((n_chunks, P, IDX_COLS), dtype=np.int16)
        for ci, ch in enumerate(chunks):
            g0, nt = ch["g0"], ch["nt"]
            src_idx[ci] = _wrap_idx(src_rel[g0: g0 + nt].reshape(nt * P))

        # one-hot S2 [node, T*128 edge slots]; PAD_DSTL rows match nothing
        s2 = (np.arange(P, dtype=np.float32)[:, None]
              == dstl.reshape(T * P)[None, :]).astype(s2_dt)
        # scatter one-hot S [edge-partition, T*128 node cols]:
        # st[e, t*128+n] = (dstl[t, e] == n)
        st = (dstl.T[:, :, None]
              == np.arange(P, dtype=np.float32)[None, None, :]).astype(s2_dt)
        # ew replicated per head: [128, T*8]
        ew8 = np.repeat(eww.T[:, :, None], H, axis=2).reshape(P, T * H)

        per_core.append(dict(
            src_idx=src_idx,
            ew8=np.ascontiguousarray(ew8).astype(NP_BF),       # [128, T*8]
            s2=np.ascontiguousarray(s2),                       # [128, T*128]
            st=np.ascontiguousarray(st.reshape(P, T * P)),     # [128, T*128]
        ))

    return plan, per_core


# build stages for HW bisection: 1=phase1+table only, 2=+gathers+s2,
# 3=+score/rhs pipeline, 4=full (default)
BUILD_STAGE = 4
# repeat whole kernel body inside one NEFF (for timing by differencing)
REPS = 1


# ---------------------------------------------------------------- builder
def build(plan):
    n_chunks = plan["n_chunks"]
    chunks = plan["chunks"]
    T = plan["T"]
    tile_block = plan["tile_block"]

    nc = bacc.Bacc("TRN2", target_bir_lowering=False, debug=False,
                   num_devices=NCORES, num_swdge_queues=4)
    qctr = [0]
    S2DT = F8 if USE_FP8_S2 else BF

    # inputs
    xt_in = nc.dram_tensor("xt_in", [P, NPAD], BF, kind="ExternalInput")
    rhsw_in = nc.dram_tensor("rhsw_in", [P, IN_F + 2 * H], BF,
                             kind="ExternalInput")
    epw_in = nc.dram_tensor("epw_in", [P, H], BF, kind="ExternalInput")
    epb_in = nc.dram_tensor("epb_in", [P, H], BF, kind="ExternalInput")
    bias_in = nc.dram_tensor("bias_in", [P, OUT_F], FP, kind="ExternalInput")
    ew8_in = nc.dram_tensor("ew8_in", [P, T * H], BF, kind="ExternalInput")
    sidx_in = nc.dram_tensor("sidx_in", [n_chunks, P, IDX_COLS],
                             mybir.dt.int16, kind="ExternalInput")
    s2_in = nc.dram_tensor("s2_in", [P, T * P], S2DT, kind="ExternalInput")
    st_in = nc.dram_tensor("st_in", [P, T * P], S2DT, kind="ExternalInput")
    out = nc.dram_tensor("out", [NPC, OUT_F], FP, kind="ExternalOutput")

    with tile.TileContext(nc) as tc:
        for _rep in range(REPS):
            with tc.tile_pool(name="dram", bufs=1, space="DRAM") as dram, \
                 tc.tile_pool(name="statics", bufs=1) as statics:

                hs_A = dram.tile([HALF, ROW], BF)
                hs_B = dram.tile([HALF, ROW], BF)

                # ---------------- statics
                epw_sb = statics.tile([P, H], BF)
                nc.sync.dma_start(epw_sb[:], epw_in[:])
                epb_sb = statics.tile([P, H], BF)
                nc.sync.dma_start(epb_sb[:], epb_in[:])
                bias_sb = statics.tile([P, OUT_F], FP)
                nc.sync.dma_start(bias_sb[:], bias_in[:])
                rhsw_sb = statics.tile([P, IN_F + 2 * H], BF)
                nc.sync.dma_start(rhsw_sb[:], rhsw_in[:])
                # attn_d for this core's own 49 blocks (rotated blocks 0..48)
                ad_blk = statics.tile([P, NBLK * H], BF)

                # ---------------- phase 1: full node table, local
                with tc.tile_pool(name="p1x", bufs=2) as p1x, \
                     tc.tile_pool(name="p1s", bufs=3) as p1s, \
                     tc.tile_pool(name="p1ps", bufs=4, space="PSUM") as p1ps:
                    for ch in range(NXCH):
                        xc = p1x.tile([P, XBLK * P], BF, tag="xc")
                        nc.sync.dma_start(
                            xc[:], xt_in[:, ch * XBLK * P:
                                         (ch + 1) * XBLK * P])
                        stage = p1s.tile([P, XBLK * ROW], BF, tag="stage")
                        CW = IN_F + 2 * H
                        for k in range(0, XBLK, 2):
                            g = ch * XBLK + k
                            hpsum = p1ps.tile([P, 2 * CW], FP, space="PSUM")
                            for j in range(2):
                                nc.tensor.matmul(
                                    out=hpsum[:, j * CW: (j + 1) * CW],
                                    lhsT=xc[:, (k + j) * P: (k + j + 1) * P],
                                    rhs=rhsw_sb[:], start=True, stop=True)
                            # [ad | h | as] -> row [h|as], own-slice ad kept
                            nc.scalar.activation(
                                stage[:, k * ROW: (k + 2) * ROW].rearrange(
                                    "p (t f) -> p t f", f=ROW)[:, :, 0:IN_F + H],
                                hpsum[:].rearrange("p (t f) -> p t f",
                                                   f=CW)[:, :, H: 2 * H + IN_F],
                                mybir.ActivationFunctionType.Copy)
                            nown = max(0, min(NBLK - g, 2))
                            if nown:
                                nc.scalar.activation(
                                    ad_blk[:, g * H: (g + nown) * H],
                                    hpsum[:].rearrange(
                                        "p (t f) -> p t f",
                                        f=CW)[:, :nown, 0:H],
                                    mybir.ActivationFunctionType.Copy)
                        half_t = hs_A if ch < NXCH // 2 else hs_B
                        r0 = (ch % (NXCH // 2)) * XBLK * P
                        nc.sync.dma_start(
                            half_t[r0: r0 + XBLK * P, :].rearrange(
                                "(t p) f -> p t f", p=P),
                            stage[:].rearrange("p (t f) -> p t f", f=ROW))

                # ---------------- phase 2
                with tc.tile_pool(name="meta", bufs=1) as meta, \
                     tc.tile_pool(name="gp", bufs=3) as gp, \
                     tc.tile_pool(name="s2p", bufs=3) as s2p, \
                     tc.tile_pool(name="sp", bufs=3) as sp, \
                     tc.tile_pool(name="rp", bufs=3) as rp, \
                     tc.tile_pool(name="ep", bufs=3) as ep, \
                     tc.tile_pool(name="op", bufs=3) as opool, \
                     tc.tile_pool(name="adps", bufs=2, space="PSUM") as adps, \
                     tc.tile_pool(name="bps", bufs=4, space="PSUM") as bps:

                    ew8_sb = meta.tile([P, T * H], BF)
                    nc.sync.dma_start(ew8_sb[:], ew8_in[:])
                    sidx_all = meta.tile([P, n_chunks, IDX_COLS],
                                         mybir.dt.int16)
                    nc.sync.dma_start(
                        sidx_all[:],
                        sidx_in[:].rearrange("c p i -> p c i"))

                    # e4_all = ew*epw + epb for every slot, upfront
                    e4_all = meta.tile([P, T * H], BF)
                    e4v_all = e4_all[:].rearrange("p (t h) -> p t h", h=H)
                    nc.vector.tensor_tensor(
                        out=e4v_all,
                        in0=ew8_sb[:].rearrange("p (t h) -> p t h", h=H),
                        in1=epw_sb[:].unsqueeze(1).broadcast_to([P, T, H]),
                        op=mybir.AluOpType.mult)
                    nc.vector.tensor_tensor(
                        out=e4v_all, in0=e4v_all,
                        in1=epb_sb[:].unsqueeze(1).broadcast_to([P, T, H]),
                        op=mybir.AluOpType.add)

                    chunk_tiles = {}

                    def emit_chunk(ci):
                        ch = chunks[ci]
                        g0, nt = ch["g0"], ch["nt"]
                        nidx = nt * P
                        n16 = nidx // 16
                        if BUILD_STAGE == 1:
                            return

                        gbuf = gp.tile([P, CHUNK_TILES, ROW], BF, tag="gbuf")
                        half_ap = (hs_A[:] if ch["stream"] == 0 else hs_B[:])
                        nc.gpsimd.dma_gather(
                            out_ap=gbuf[:, :nt, :], in_ap=half_ap,
                            idxs_ap=sidx_all[:, ci, :n16],
                            num_idxs=nidx, num_idxs_reg=nidx, elem_size=ROW,
                            single_packet=False, queue_num=qctr[0] % 4)
                        qctr[0] += 1

                        s2c = s2p.tile([P, CHUNK_TILES * P], S2DT, tag="s2c")
                        nc.sync.dma_start(
                            s2c[:, : nt * P],
                            s2_in[:, g0 * P: (g0 + nt) * P])
                        if BUILD_STAGE == 2:
                            chunk_tiles[ci] = (gbuf, s2c)
                            return

                        # ad_edge[e, h] per tile via one-hot matmul
                        adp = adps.tile([P, CHUNK_TILES * H], FP,
                                        space="PSUM", tag="adp")
                        for t in range(nt):
                            b = int(tile_block[g0 + t])
                            nc.tensor.matmul(
                                out=adp[:, t * H: (t + 1) * H],
                                lhsT=s2c[:, t * P: (t + 1) * P],
                                rhs=ad_blk[:, b * H: (b + 1) * H],
                                start=True, stop=True)

                        # one-hot S [P, nt*128] shipped fp8 from host
                        s_t = sp.tile([P, CHUNK_TILES * P], S2DT, tag="s_t")
                        nc.sync.dma_start(
                            s_t[:, : nt * P],
                            st_in[:, g0 * P: (g0 + nt) * P])

                        # scores: e0 = as + ad ; e2 = lrelu(e0) ;
                        # e5 = e2 + e4 ; p = exp(e5)
                        e0 = ep.tile([P, CHUNK_TILES * H], BF, tag="e0")
                        e0v = e0[:].rearrange("p (t h) -> p t h",
                                              h=H)[:, :nt, :]
                        nc.vector.tensor_tensor(
                            out=e0v, in0=gbuf[:, :nt, AS_OFF: AS_OFF + H],
                            in1=adp[:, : nt * H].rearrange(
                                "p (t h) -> p t h", h=H),
                            op=mybir.AluOpType.add)
                        e1 = ep.tile([P, CHUNK_TILES * H], BF, tag="e1")
                        e1v = e1[:].rearrange("p (t h) -> p t h",
                                              h=H)[:, :nt, :]
                        nc.vector.tensor_scalar_mul(out=e1v, in0=e0v,
                                                    scalar1=ALPHA)
                        e2 = ep.tile([P, CHUNK_TILES * H], BF, tag="e2")
                        e2v = e2[:].rearrange("p (t h) -> p t h",
                                              h=H)[:, :nt, :]
                        nc.vector.tensor_tensor(out=e2v, in0=e0v, in1=e1v,
                                                op=mybir.AluOpType.max)
                        e5 = ep.tile([P, CHUNK_TILES * H], BF, tag="e5")
                        e5v = e5[:].rearrange("p (t h) -> p t h",
                                              h=H)[:, :nt, :]
                        nc.vector.tensor_tensor(
                            out=e5v, in0=e2v,
                            in1=e4v_all[:, g0: g0 + nt, :],
                            op=mybir.AluOpType.add)

                        # rhs tile: [msgs(128) | p(8)] per tile
                        rhs = rp.tile([P, CHUNK_TILES * (OUT_F + H)], BF,
                                      tag="rhs")
                        rhs_v = rhs[:].rearrange("p (t f) -> p t f",
                                                 f=OUT_F + H)
                        nc.scalar.activation(
                            rhs_v[:, :nt, OUT_F: OUT_F + H], e5v,
                            mybir.ActivationFunctionType.Exp)
                        # msgs = h * p; h is d-major so innermost dim is
                        # the head axis (stride 1) for every operand -> 2x
                        nc.vector.tensor_tensor(
                            out=rhs_v[:, :nt, 0:OUT_F].rearrange(
                                "p t (d h) -> p t d h", h=H),
                            in0=gbuf[:, :nt, 0:IN_F].rearrange(
                                "p t (d h) -> p t d h", h=H),
                            in1=rhs_v[:, :nt, OUT_F: OUT_F + H].unsqueeze(2)
                                .broadcast_to([P, nt, HD, H]),
                            op=mybir.AluOpType.mult)
                        chunk_tiles[ci] = (s_t, rhs)

                    if BUILD_STAGE < 4:
                        for ci in range(n_chunks):
                            emit_chunk(ci)
                        dump = opool.tile([P, OUT_F], FP, tag="dump")
                        if BUILD_STAGE == 1:
                            nc.vector.memset(dump[:], 0.0)
                        elif BUILD_STAGE == 2:
                            g0buf = chunk_tiles[0][0]
                            nc.vector.tensor_copy(dump[:],
                                                  g0buf[:, 0, 0:OUT_F])
                        else:
                            r0 = chunk_tiles[0][1]
                            nc.vector.tensor_copy(dump[:], r0[:, 0:OUT_F])
                        for b in range(NBLK):
                            nc.sync.dma_start(out[b * P: (b + 1) * P, :],
                                              dump[:])

                    for b in range(NBLK if BUILD_STAGE >= 4 else 0):
                        tl = plan["block_tiles"][b]
                        for (ci, slot) in tl:
                            if ci not in chunk_tiles:
                                emit_chunk(ci)
                        psum_b = bps.tile([P, OUT_F + H], FP, space="PSUM",
                                          tag="psum_b")
                        for i, (ci, slot) in enumerate(tl):
                            s_t, rhs = chunk_tiles[ci]
                            nc.tensor.matmul(
                                out=psum_b[:],
                                lhsT=s_t[:, slot * P: (slot + 1) * P],
                                rhs=rhs[:, slot * (OUT_F + H):
                                        (slot + 1) * (OUT_F + H)],
                                start=(i == 0), stop=(i == len(tl) - 1))
                        # normalize + bias
                        s_eps = opool.tile([P, H], FP, tag="s_eps")
                        nc.vector.tensor_scalar_add(
                            out=s_eps[:], in0=psum_b[:, OUT_F: OUT_F + H],
                            scalar1=EPS)
                        rcp = opool.tile([P, H], FP, tag="rcp")
                        nc.vector.reciprocal(rcp[:], s_eps[:])
                        ob1 = opool.tile([P, OUT_F], FP, tag="ob1")
                        nc.vector.tensor_tensor(
                            out=ob1[:].rearrange("p (d h) -> p d h", h=H),
                            in0=psum_b[:, 0:OUT_F].rearrange(
                                "p (d h) -> p d h", h=H),
                            in1=rcp[:].unsqueeze(1).broadcast_to([P, HD, H]),
                            op=mybir.AluOpType.mult)
                        ob2 = opool.tile([P, OUT_F], FP, tag="ob2")
                        nc.vector.tensor_tensor(out=ob2[:], in0=ob1[:],
                                                in1=bias_sb[:],
                                                op=mybir.AluOpType.add)
                        nc.sync.dma_start(out[b * P: (b + 1) * P, :], ob2[:])

    nc.compile()
    # SWDGE constraint: a DMA semaphore may only be updated from one queue.
    # Tile assigns DMASW lanes post-scheduling, so align queue_num to lane.
    for f in nc.m.functions:
        for bb in f.blocks:
            for ins in bb.instructions:
                if type(ins).__name__ == "InstDMAGatherAnt":
                    si = ins.sync_info
                    lane = None
                    for u in si.on_update:
                        nm = u.ant_name or ""
                        if nm.startswith("DMASW"):
                            lane = int(nm[5:].split("_")[0])
                            break
                    assert lane is not None, "gather without DMASW sem"
                    ins.queue_num = lane % 4
    return nc


# ---------------------------------------------------------------- host API
def make_in_maps(x, W, a_src, a_dst, ep_w, ep_b, bias, per_core):
    x = np.asarray(x, dtype=np.float32)
    W = np.asarray(W, dtype=np.float32)
    a_src = np.asarray(a_src, dtype=np.float32)
    a_dst = np.asarray(a_dst, dtype=np.float32)
    ep_w = np.asarray(ep_w, dtype=np.float32)
    ep_b = np.asarray(ep_b, dtype=np.float32)
    bias = np.asarray(bias, dtype=np.float32)

    x_pad = np.zeros((NPAD, IN_F), dtype=np.float32)
    x_pad[:N] = x
    # rhs_w = [W@a_dst | W (d-major cols) | W@a_src]  [IN, 144]
    w_flat = W.transpose(1, 2, 0).reshape(IN_F, HD * H)       # [IN, d*8+h]
    wad = np.einsum('hid,hd->ih', W, a_dst)                   # [IN, H]
    was = np.einsum('hid,hd->ih', W, a_src)                   # [IN, H]
    rhs_w = np.concatenate([wad, w_flat, was], axis=1).astype(NP_BF)
    bias_dm = bias.reshape(H, HD).T.reshape(OUT_F)            # d-major

    rep = lambda v, dt: np.ascontiguousarray(
        np.broadcast_to(v[None, :], (P, v.shape[0]))).astype(dt)

    maps = []
    for c in range(NCORES):
        pc = per_core[c]
        x_rot = np.roll(x_pad, -c * NPC, axis=0)
        xt = np.ascontiguousarray(x_rot.T).astype(NP_BF)
        maps.append({
            "xt_in": xt,
            "rhsw_in": np.ascontiguousarray(rhs_w),
            "epw_in": rep(ep_w, NP_BF),
            "epb_in": rep(ep_b, NP_BF),
            "bias_in": rep(bias_dm, np.float32),
            "ew8_in": pc["ew8"],
            "sidx_in": pc["src_idx"],
            "s2_in": pc["s2"],
            "st_in": pc["st"],
        })
    return maps


_CACHE = {}


def kernel(x, edge_index, edge_weight, W, a_src, a_dst, ep_w, ep_b, bias):
    import hashlib
    key = hashlib.sha1(
        np.ascontiguousarray(np.asarray(edge_index, dtype=np.int64))
    ).hexdigest()
    if key not in _CACHE:
        plan, per_core = plan_and_inputs(edge_index, edge_weight)
        nc = build(plan)
        _CACHE[key] = (plan, per_core, nc)
    plan, per_core, nc = _CACHE[key]

    in_maps = make_in_maps(x, W, a_src, a_dst, ep_w, ep_b, bias, per_core)
    res = run_bass_kernel_spmd(nc, in_maps, core_ids=list(range(NCORES)),
                               trace=False)
    out_full = np.empty((NPAD, OUT_F), dtype=np.float32)
    for c in range(NCORES):
        out_full[c * NPC: (c + 1) * NPC] = res.results[c]["out"]
    # device columns are d-major (d*8+h); back to h*16+d
    out_full = out_full.reshape(-1, HD, H).transpose(0, 2, 1).reshape(
        -1, OUT_F)
    return out_full[:N]


# revision 6
# speedup vs baseline: 1.0076x; 1.0076x over previous
"""EnhancedCorrelationGNN Trainium2 kernel (8 NeuronCores, SPMD).

Strategy: destination-sorted edge processing with node-range output sharding,
fully collective-free.
 - Host (free): counting-sort edges by dst, partition nodes into 8 ranges of
   6272 (49 blocks x 128 nodes per core). Per core the node table is ROTATED
   so its own slice comes first; per block, edges are split by rotated src
   half (dma_gather int16 index limit) and padded to 128-edge tiles with
   cross-core-uniform tile counts (one SPMD program).
 - Phase 1 (device): EVERY core computes the FULL node table from the
   replicated x input: h = x @ W plus both attention projections in ONE bf16
   matmul per 128-node tile (rhs = [W@a_dst | W | W@a_src] prepped on host),
   writes bf16 [h|as] rows (512B) to local DRAM. attn_d for the core's own
   49 blocks stays in SBUF. No AllGather.
 - Phase 2 (device): per 32-tile chunk, one dma_gather of bf16 [h|as] rows
   by src (512B/edge); attn_d is expanded per-edge by a TensorE matmul with
   a host-shipped fp8 one-hot (node x edge) instead of a second gather.
   Scores: DVE adds + ACT Lrelu/Exp; messages bf16; one-hot segment matrix
   via is_equal(dstl, iota) in bf16; per-tile bf16 TensorE matmul
   scatter-accumulates [msgs | p] into the block PSUM; per block normalize
   by 1/(sum p + 1e-10), add bias, DMA out.
"""
import sys

if "/opt/trn_rl_repo" not in sys.path:
    sys.path.insert(0, "/opt/trn_rl_repo")

import numpy as np
import ml_dtypes

import concourse.bass as bass
import concourse.bacc as bacc
import concourse.mybir as mybir
import concourse.tile as tile
from concourse.bass_utils import run_bass_kernel_spmd

# ---------------------------------------------------------------- constants
N = 50000
E = 800000
IN_F = 128
H = 8
HD = 16
OUT_F = H * HD          # 128
ALPHA = 0.2
EPS = 1e-10

NCORES = 8
P = 128
NPC = 6272              # nodes per core = 49 * 128; 8*6272 = 50176 >= N
NPAD = NCORES * NPC     # 50176
NBLK = NPC // P         # 49
HALF = NPAD // 2        # 25088 rotated-table rows per gather stream

ROW = 256               # table row elems (bf16): h(128) | as(8) | pad -> 512B
AS_OFF = 128            # attn_s offset within row
CHUNK_TILES = 32        # tiles per gather/DVE chunk
IDX_COLS = CHUNK_TILES * P // 16   # wrapped int16 idx columns per chunk
PAD_DSTL = 300.0        # one-hot miss sentinel (exact in bf16)
XBLK = 28               # phase-1 blocks per xT chunk; 392 = 14 * 28
NXCH = (NPAD // P) // XBLK         # 14 phase-1 chunks (7 per half)

FP = mybir.dt.float32
BF = mybir.dt.bfloat16
F8 = mybir.dt.float8e4
NP_BF = ml_dtypes.bfloat16
NP_F8 = ml_dtypes.float8_e4m3

USE_FP8_S2 = True       # one-hot S2 dtype (fp8 halves its DMA vs bf16)


# ---------------------------------------------------------------- planning
def _cdiv(a, b):
    return -(-a // b)


def _wrap_idx(idx_flat: np.ndarray) -> np.ndarray:
    """[n] -> [128, IDX_COLS] int16: idx j at [j%16, j//16], replicated x8."""
    n = idx_flat.shape[0]
    assert n % 16 == 0
    w = idx_flat.reshape(n // 16, 16).T.astype(np.int16)      # [16, n/16]
    w = np.tile(w, (8, 1))                                    # [128, n/16]
    out = np.zeros((P, IDX_COLS), dtype=np.int16)
    out[:, : w.shape[1]] = w
    return out


def plan_and_inputs(edge_index, edge_weight):
    """Host-side edge partitioning. Returns (plan, per_core_arrays).

    plan (core-independent, defines the SPMD program):
      KA, KB: [NBLK] tiles per (block, half)
      chunks: list of dicts(stream, g0, nt) over stream-major tile ids
      block_tiles: per block, list of (chunk_id, slot) in matmul order
      tile_block: [T] block id of each global tile
      T, n_chunks
    per_core_arrays[c]:
      src_idx [n_chunks,128,IDX_COLS] i16 (stream-relative, rotated table)
      dstl    [128, T] bf16 (block-relative dst, PAD_DSTL for pad slots)
      ew      [128, T] bf16
      s2      [128, T*128] fp8/bf16 one-hot: s2[n, t*128+e] = (dstl[e,t]==n)
    """
    src = np.asarray(edge_index[0], dtype=np.int64)
    dst = np.asarray(edge_index[1], dtype=np.int64)
    ew = np.asarray(edge_weight, dtype=np.float32)

    order = np.argsort(dst, kind="stable")
    src_s, dst_s, ew_s = src[order], dst[order], ew[order]

    # block boundaries over sorted dst
    blk_starts = np.searchsorted(dst_s, np.arange(0, NPAD + 1, P))
    # per (core, block, half) edge index lists (into the sorted arrays)
    cnt = np.zeros((NCORES, NBLK, 2), dtype=np.int64)
    lists = [[[None, None] for _ in range(NBLK)] for _ in range(NCORES)]
    rot_all = []
    for c in range(NCORES):
        rot = (src_s - c * NPC) % NPAD     # rotated src row per core
        rot_all.append(rot)
        for b in range(NBLK):
            g = c * NBLK + b
            lo, hi = blk_starts[g], blk_starts[g + 1]
            r = rot[lo:hi]
            mA = r < HALF
            idxs = np.arange(lo, hi)
            lists[c][b][0] = idxs[mA]
            lists[c][b][1] = idxs[~mA]
            cnt[c, b, 0] = mA.sum()
            cnt[c, b, 1] = (~mA).sum()

    KA = np.maximum(_cdiv(cnt[:, :, 0].max(axis=0), P), 1).astype(np.int64)
    KB = _cdiv(cnt[:, :, 1].max(axis=0), P).astype(np.int64)

    T_A = int(KA.sum())
    T_B = int(KB.sum())
    T = T_A + T_B
    cumKA = np.concatenate([[0], np.cumsum(KA)])
    cumKB = np.concatenate([[0], np.cumsum(KB)])

    # chunks: stream-major [0,T_A) then [T_A,T)
    chunks = []
    g = 0
    while g < T_A:
        nt = min(CHUNK_TILES, T_A - g)
        chunks.append(dict(stream=0, g0=g, nt=nt))
        g += nt
    while g < T:
        nt = min(CHUNK_TILES, T - g)
        chunks.append(dict(stream=1, g0=g, nt=nt))
        g += nt
    n_chunks = len(chunks)

    chunk_of = np.empty(T, dtype=np.int64)
    slot_of = np.empty(T, dtype=np.int64)
    for ci, ch in enumerate(chunks):
        chunk_of[ch["g0"]: ch["g0"] + ch["nt"]] = ci
        slot_of[ch["g0"]: ch["g0"] + ch["nt"]] = np.arange(ch["nt"])

    tile_block = np.empty(T, dtype=np.int64)
    block_tiles = []
    for b in range(NBLK):
        tl = []
        for k in range(KA[b]):
            gidx = cumKA[b] + k
            tile_block[gidx] = b
            tl.append((int(chunk_of[gidx]), int(slot_of[gidx])))
        for k in range(KB[b]):
            gidx = T_A + cumKB[b] + k
            tile_block[gidx] = b
            tl.append((int(chunk_of[gidx]), int(slot_of[gidx])))
        block_tiles.append(tl)

    plan = dict(KA=KA, KB=KB, T=T, T_A=T_A, chunks=chunks,
                block_tiles=block_tiles, tile_block=tile_block,
                n_chunks=n_chunks)

    # ---------------- per-core slot arrays
    s2_dt = NP_F8 if USE_FP8_S2 else NP_BF
    per_core = []
    for c in range(NCORES):
        src_rel = np.zeros((T, P), dtype=np.int16)
        dst_rel = np.zeros((T, P), dtype=np.int16)
        dstl = np.full((T, P), PAD_DSTL, dtype=np.float32)
        eww = np.zeros((T, P), dtype=np.float32)
        rot = rot_all[c]
        for b in range(NBLK):
            for half, K, cum, base in ((0, KA, cumKA, 0),
                                       (1, KB, cumKB, T_A)):
                idxs = lists[c][b][half]
                # src-sorted slots: consecutive gather rows are nearly
                # contiguous in DRAM (page locality)
                idxs = idxs[np.argsort(rot[idxs], kind="stable")]
                n = idxs.shape[0]
                g0 = base + cum[b]
                nslots = int(K[b]) * P
                s_loc = np.zeros(nslots, dtype=np.int64)
                d_loc = np.zeros(nslots, dtype=np.int64)
                dl = np.full(nslots, PAD_DSTL, dtype=np.float32)
                w = np.zeros(nslots, dtype=np.float32)
                if n:
                    s_loc[:n] = rot[idxs] - (HALF if half else 0)
                    d_loc[:n] = dst_s[idxs] - c * NPC
                    dl[:n] = (dst_s[idxs] - (c * NPC + b * P)).astype(
                        np.float32)
                    w[:n] = ew_s[idxs]
                src_rel[g0: g0 + int(K[b])] = s_loc.reshape(
                    int(K[b]), P).astype(np.int16)
                dst_rel[g0: g0 + int(K[b])] = d_loc.reshape(
                    int(K[b]), P).astype(np.int16)
                dstl[g0: g0 + int(K[b])] = dl.reshape(int(K[b]), P)
                eww[g0: g0 + int(K[b])] = w.reshape(int(K[b]), P)

        src_idx = np.zeros((n_chunks, P, IDX_COLS), dtype=np.int16)
        dst_idx = np.zeros((n_chunks, P, IDX_COLS), dtype=np.int16)
        for ci, ch in enumerate(chunks):
            g0, nt = ch["g0"], ch["nt"]
            src_idx[ci] = _wrap_idx(src_rel[g0: g0 + nt].reshape(nt * P))
            dst_idx[ci] = _wrap_idx(dst_rel[g0: g0 + nt].reshape(nt * P))

        # scatter one-hot S [edge-partition, T*128 node cols]:
        # st[e, t*128+n] = (dstl[t, e] == n)
        st = (dstl.T[:, :, None]
              == np.arange(P, dtype=np.float32)[None, None, :]).astype(s2_dt)
        # ew replicated per head: [128, T*8]
        ew8 = np.repeat(eww.T[:, :, None], H, axis=2).reshape(P, T * H)

        per_core.append(dict(
            src_idx=src_idx, dst_idx=dst_idx,
            ew8=np.ascontiguousarray(ew8).astype(NP_BF),       # [128, T*8]
            st=np.ascontiguousarray(st.reshape(P, T * P)),     # [128, T*128]
        ))

    return plan, per_core


# build stages for HW bisection: 1=phase1+table only, 2=+gathers+s2,
# 3=+score/rhs pipeline, 4=full (default)
BUILD_STAGE = 4
# repeat whole kernel body inside one NEFF (for timing by differencing)
REPS = 1


# ---------------------------------------------------------------- builder
def build(plan):
    n_chunks = plan["n_chunks"]
    chunks = plan["chunks"]
    T = plan["T"]
    tile_block = plan["tile_block"]

    nc = bacc.Bacc("TRN2", target_bir_lowering=False, debug=False,
                   num_devices=NCORES, num_swdge_queues=4)
    qctr = [0]
    S2DT = F8 if USE_FP8_S2 else BF

    # inputs
    xt_in = nc.dram_tensor("xt_in", [P, NPAD], BF, kind="ExternalInput")
    rhsw_in = nc.dram_tensor("rhsw_in", [P, IN_F + 2 * H], BF,
                             kind="ExternalInput")
    epw_in = nc.dram_tensor("epw_in", [P, H], BF, kind="ExternalInput")
    epb_in = nc.dram_tensor("epb_in", [P, H], BF, kind="ExternalInput")
    bias_in = nc.dram_tensor("bias_in", [P, OUT_F], FP, kind="ExternalInput")
    ew8_in = nc.dram_tensor("ew8_in", [P, T * H], BF, kind="ExternalInput")
    sidx_in = nc.dram_tensor("sidx_in", [n_chunks, P, IDX_COLS],
                             mybir.dt.int16, kind="ExternalInput")
    didx_in = nc.dram_tensor("didx_in", [n_chunks, P, IDX_COLS],
                             mybir.dt.int16, kind="ExternalInput")
    st_in = nc.dram_tensor("st_in", [P, T * P], S2DT, kind="ExternalInput")
    out = nc.dram_tensor("out", [NPC, OUT_F], FP, kind="ExternalOutput")

    with tile.TileContext(nc) as tc:
        for _rep in range(REPS):
            with tc.tile_pool(name="dram", bufs=1, space="DRAM") as dram, \
                 tc.tile_pool(name="statics", bufs=1) as statics:

                hs_A = dram.tile([HALF, ROW], BF)
                hs_B = dram.tile([HALF, ROW], BF)
                ad_pad = dram.tile([NPC, P], BF)

                # ---------------- statics
                epw_sb = statics.tile([P, H], BF)
                nc.sync.dma_start(epw_sb[:], epw_in[:])
                epb_sb = statics.tile([P, H], BF)
                nc.sync.dma_start(epb_sb[:], epb_in[:])
                bias_sb = statics.tile([P, OUT_F], FP)
                nc.sync.dma_start(bias_sb[:], bias_in[:])
                rhsw_sb = statics.tile([P, IN_F + 2 * H], BF)
                nc.sync.dma_start(rhsw_sb[:], rhsw_in[:])
                # attn_d for this core's own 49 blocks (rotated blocks 0..48)
                ad_blk = statics.tile([P, NBLK * H], BF)

                # ---------------- phase 1: full node table, local
                with tc.tile_pool(name="p1x", bufs=2) as p1x, \
                     tc.tile_pool(name="p1s", bufs=3) as p1s, \
                     tc.tile_pool(name="p1ps", bufs=4, space="PSUM") as p1ps:
                    for ch in range(NXCH):
                        xc = p1x.tile([P, XBLK * P], BF, tag="xc")
                        nc.sync.dma_start(
                            xc[:], xt_in[:, ch * XBLK * P:
                                         (ch + 1) * XBLK * P])
                        stage = p1s.tile([P, XBLK * ROW], BF, tag="stage")
                        CW = IN_F + 2 * H
                        for k in range(0, XBLK, 2):
                            g = ch * XBLK + k
                            hpsum = p1ps.tile([P, 2 * CW], FP, space="PSUM")
                            for j in range(2):
                                nc.tensor.matmul(
                                    out=hpsum[:, j * CW: (j + 1) * CW],
                                    lhsT=xc[:, (k + j) * P: (k + j + 1) * P],
                                    rhs=rhsw_sb[:], start=True, stop=True)
                            # [ad | h | as] -> row [h|as], own-slice ad kept
                            nc.scalar.activation(
                                stage[:, k * ROW: (k + 2) * ROW].rearrange(
                                    "p (t f) -> p t f", f=ROW)[:, :, 0:IN_F + H],
                                hpsum[:].rearrange("p (t f) -> p t f",
                                                   f=CW)[:, :, H: 2 * H + IN_F],
                                mybir.ActivationFunctionType.Copy)
                            nown = max(0, min(NBLK - g, 2))
                            if nown:
                                nc.scalar.activation(
                                    ad_blk[:, g * H: (g + nown) * H],
                                    hpsum[:].rearrange(
                                        "p (t f) -> p t f",
                                        f=CW)[:, :nown, 0:H],
                                    mybir.ActivationFunctionType.Copy)
                        half_t = hs_A if ch < NXCH // 2 else hs_B
                        r0 = (ch % (NXCH // 2)) * XBLK * P
                        nc.sync.dma_start(
                            half_t[r0: r0 + XBLK * P, :].rearrange(
                                "(t p) f -> p t f", p=P),
                            stage[:].rearrange("p (t f) -> p t f", f=ROW))

                # own-slice attn_d rows -> DRAM table (gathered per edge)
                nc.sync.dma_start(
                    ad_pad[:].rearrange("(t p) f -> p t f", p=P)[:, :, 0:H],
                    ad_blk[:].rearrange("p (t h) -> p t h", h=H))

                # ---------------- phase 2
                with tc.tile_pool(name="meta", bufs=1) as meta, \
                     tc.tile_pool(name="gp", bufs=3) as gp, \
                     tc.tile_pool(name="s2p", bufs=3) as s2p, \
                     tc.tile_pool(name="sp", bufs=3) as sp, \
                     tc.tile_pool(name="rp", bufs=3) as rp, \
                     tc.tile_pool(name="ep", bufs=3) as ep, \
                     tc.tile_pool(name="op", bufs=3) as opool, \
                         tc.tile_pool(name="bps", bufs=4, space="PSUM") as bps:

                    ew8_sb = meta.tile([P, T * H], BF)
                    nc.sync.dma_start(ew8_sb[:], ew8_in[:])
                    sidx_all = meta.tile([P, n_chunks, IDX_COLS],
                                         mybir.dt.int16)
                    nc.sync.dma_start(
                        sidx_all[:],
                        sidx_in[:].rearrange("c p i -> p c i"))
                    didx_all = meta.tile([P, n_chunks, IDX_COLS],
                                         mybir.dt.int16)
                    nc.sync.dma_start(
                        didx_all[:],
                        didx_in[:].rearrange("c p i -> p c i"))

                    # e4_all = ew*epw + epb for every slot, upfront
                    e4_all = meta.tile([P, T * H], BF)
                    e4v_all = e4_all[:].rearrange("p (t h) -> p t h", h=H)
                    nc.vector.tensor_tensor(
                        out=e4v_all,
                        in0=ew8_sb[:].rearrange("p (t h) -> p t h", h=H),
                        in1=epw_sb[:].unsqueeze(1).broadcast_to([P, T, H]),
                        op=mybir.AluOpType.mult)
                    nc.vector.tensor_tensor(
                        out=e4v_all, in0=e4v_all,
                        in1=epb_sb[:].unsqueeze(1).broadcast_to([P, T, H]),
                        op=mybir.AluOpType.add)

                    chunk_tiles = {}

                    def emit_chunk(ci):
                        ch = chunks[ci]
                        g0, nt = ch["g0"], ch["nt"]
                        nidx = nt * P
                        n16 = nidx // 16
                        if BUILD_STAGE == 1:
                            return

                        gbuf = gp.tile([P, CHUNK_TILES, ROW], BF, tag="gbuf")
                        half_ap = (hs_A[:] if ch["stream"] == 0 else hs_B[:])
                        nc.gpsimd.dma_gather(
                            out_ap=gbuf[:, :nt, :], in_ap=half_ap,
                            idxs_ap=sidx_all[:, ci, :n16],
                            num_idxs=nidx, num_idxs_reg=nidx, elem_size=ROW,
                            single_packet=False, queue_num=qctr[0] % 4)
                        qctr[0] += 1

                        adbuf = s2p.tile([P, CHUNK_TILES, P], BF,
                                         tag="adbuf")
                        nc.gpsimd.dma_gather(
                            out_ap=adbuf[:, :nt, :], in_ap=ad_pad[:],
                            idxs_ap=didx_all[:, ci, :n16],
                            num_idxs=nidx, num_idxs_reg=nidx, elem_size=P,
                            single_packet=False, queue_num=qctr[0] % 4)
                        qctr[0] += 1
                        if BUILD_STAGE == 2:
                            chunk_tiles[ci] = (gbuf, adbuf)
                            return

                        # one-hot S [P, nt*128] shipped fp8 from host
                        s_t = sp.tile([P, CHUNK_TILES * P], S2DT, tag="s_t")
                        nc.sync.dma_start(
                            s_t[:, : nt * P],
                            st_in[:, g0 * P: (g0 + nt) * P])

                        # scores: e0 = as + ad ; e2 = lrelu(e0) ;
                        # e5 = e2 + e4 ; p = exp(e5)
                        e0 = ep.tile([P, CHUNK_TILES * H], BF, tag="e0")
                        e0v = e0[:].rearrange("p (t h) -> p t h",
                                              h=H)[:, :nt, :]
                        nc.vector.tensor_tensor(
                            out=e0v, in0=gbuf[:, :nt, AS_OFF: AS_OFF + H],
                            in1=adbuf[:, :nt, 0:H],
                            op=mybir.AluOpType.add)
                        e1 = ep.tile([P, CHUNK_TILES * H], BF, tag="e1")
                        e1v = e1[:].rearrange("p (t h) -> p t h",
                                              h=H)[:, :nt, :]
                        nc.vector.tensor_scalar_mul(out=e1v, in0=e0v,
                                                    scalar1=ALPHA)
                        e2 = ep.tile([P, CHUNK_TILES * H], BF, tag="e2")
                        e2v = e2[:].rearrange("p (t h) -> p t h",
                                              h=H)[:, :nt, :]
                        nc.vector.tensor_tensor(out=e2v, in0=e0v, in1=e1v,
                                                op=mybir.AluOpType.max)
                        e5 = ep.tile([P, CHUNK_TILES * H], BF, tag="e5")
                        e5v = e5[:].rearrange("p (t h) -> p t h",
                                              h=H)[:, :nt, :]
                        nc.vector.tensor_tensor(
                            out=e5v, in0=e2v,
                            in1=e4v_all[:, g0: g0 + nt, :],
                            op=mybir.AluOpType.add)

                        # rhs tile: [msgs(128) | p(8)] per tile
                        rhs = rp.tile([P, CHUNK_TILES * (OUT_F + H)], BF,
                                      tag="rhs")
                        rhs_v = rhs[:].rearrange("p (t f) -> p t f",
                                                 f=OUT_F + H)
                        nc.scalar.activation(
                            rhs_v[:, :nt, OUT_F: OUT_F + H], e5v,
                            mybir.ActivationFunctionType.Exp)
                        # msgs = h * p; h is d-major so innermost dim is
                        # the head axis (stride 1) for every operand -> 2x
                        nc.vector.tensor_tensor(
                            out=rhs_v[:, :nt, 0:OUT_F].rearrange(
                                "p t (d h) -> p t d h", h=H),
                            in0=gbuf[:, :nt, 0:IN_F].rearrange(
                                "p t (d h) -> p t d h", h=H),
                            in1=rhs_v[:, :nt, OUT_F: OUT_F + H].unsqueeze(2)
                                .broadcast_to([P, nt, HD, H]),
                            op=mybir.AluOpType.mult)
                        chunk_tiles[ci] = (s_t, rhs)

                    if BUILD_STAGE < 4:
                        for ci in range(n_chunks):
                            emit_chunk(ci)
                        dump = opool.tile([P, OUT_F], FP, tag="dump")
                        if BUILD_STAGE == 1:
                            nc.vector.memset(dump[:], 0.0)
                        elif BUILD_STAGE == 2:
                            g0buf = chunk_tiles[0][0]
                            nc.vector.tensor_copy(dump[:],
                                                  g0buf[:, 0, 0:OUT_F])
                        else:
                            r0 = chunk_tiles[0][1]
                            nc.vector.tensor_copy(dump[:], r0[:, 0:OUT_F])
                        for b in range(NBLK):
                            nc.sync.dma_start(out[b * P: (b + 1) * P, :],
                                              dump[:])

                    for b in range(NBLK if BUILD_STAGE >= 4 else 0):
                        tl = plan["block_tiles"][b]
                        for (ci, slot) in tl:
                            if ci not in chunk_tiles:
                                emit_chunk(ci)
                        psum_b = bps.tile([P, OUT_F + H], FP, space="PSUM",
                                          tag="psum_b")
                        for i, (ci, slot) in enumerate(tl):
                            s_t, rhs = chunk_tiles[ci]
                            nc.tensor.matmul(
                                out=psum_b[:],
                                lhsT=s_t[:, slot * P: (slot + 1) * P],
                                rhs=rhs[:, slot * (OUT_F + H):
                                        (slot + 1) * (OUT_F + H)],
                                start=(i == 0), stop=(i == len(tl) - 1))
                        # normalize + bias
                        s_eps = opool.tile([P, H], FP, tag="s_eps")
                        nc.vector.tensor_scalar_add(
                            out=s_eps[:], in0=psum_b[:, OUT_F: OUT_F + H],
                            scalar1=EPS)
                        rcp = opool.tile([P, H], FP, tag="rcp")
                        nc.vector.reciprocal(rcp[:], s_eps[:])
                        ob1 = opool.tile([P, OUT_F], FP, tag="ob1")
                        nc.vector.tensor_tensor(
                            out=ob1[:].rearrange("p (d h) -> p d h", h=H),
                            in0=psum_b[:, 0:OUT_F].rearrange(
                                "p (d h) -> p d h", h=H),
                            in1=rcp[:].unsqueeze(1).broadcast_to([P, HD, H]),
                            op=mybir.AluOpType.mult)
                        ob2 = opool.tile([P, OUT_F], FP, tag="ob2")
                        nc.vector.tensor_tensor(out=ob2[:], in0=ob1[:],
                                                in1=bias_sb[:],
                                                op=mybir.AluOpType.add)
                        nc.sync.dma_start(out[b * P: (b + 1) * P, :], ob2[:])

    nc.compile()
    # SWDGE constraint: a DMA semaphore may only be updated from one queue.
    # Tile assigns DMASW lanes post-scheduling, so align queue_num to lane.
    for f in nc.m.functions:
        for bb in f.blocks:
            for ins in bb.instructions:
                if type(ins).__name__ == "InstDMAGatherAnt":
                    si = ins.sync_info
                    lane = None
                    for u in si.on_update:
                        nm = u.ant_name or ""
                        if nm.startswith("DMASW"):
                            lane = int(nm[5:].split("_")[0])
                            break
                    assert lane is not None, "gather without DMASW sem"
                    ins.queue_num = lane % 4
    return nc


# ---------------------------------------------------------------- host API
def make_in_maps(x, W, a_src, a_dst, ep_w, ep_b, bias, per_core):
    x = np.asarray(x, dtype=np.float32)
    W = np.asarray(W, dtype=np.float32)
    a_src = np.asarray(a_src, dtype=np.float32)
    a_dst = np.asarray(a_dst, dtype=np.float32)
    ep_w = np.asarray(ep_w, dtype=np.float32)
    ep_b = np.asarray(ep_b, dtype=np.float32)
    bias = np.asarray(bias, dtype=np.float32)

    x_pad = np.zeros((NPAD, IN_F), dtype=np.float32)
    x_pad[:N] = x
    # rhs_w = [W@a_dst | W (d-major cols) | W@a_src]  [IN, 144]
    w_flat = W.transpose(1, 2, 0).reshape(IN_F, HD * H)       # [IN, d*8+h]
    wad = np.einsum('hid,hd->ih', W, a_dst)                   # [IN, H]
    was = np.einsum('hid,hd->ih', W, a_src)                   # [IN, H]
    rhs_w = np.concatenate([wad, w_flat, was], axis=1).astype(NP_BF)
    bias_dm = bias.reshape(H, HD).T.reshape(OUT_F)            # d-major

    rep = lambda v, dt: np.ascontiguousarray(
        np.broadcast_to(v[None, :], (P, v.shape[0]))).astype(dt)

    maps = []
    for c in range(NCORES):
        pc = per_core[c]
        x_rot = np.roll(x_pad, -c * NPC, axis=0)
        xt = np.ascontiguousarray(x_rot.T).astype(NP_BF)
        maps.append({
            "xt_in": xt,
            "rhsw_in": np.ascontiguousarray(rhs_w),
            "epw_in": rep(ep_w, NP_BF),
            "epb_in": rep(ep_b, NP_BF),
            "bias_in": rep(bias_dm, np.float32),
            "ew8_in": pc["ew8"],
            "sidx_in": pc["src_idx"],
            "didx_in": pc["dst_idx"],
            "st_in": pc["st"],
        })
    return maps


_CACHE = {}


def kernel(x, edge_index, edge_weight, W, a_src, a_dst, ep_w, ep_b, bias):
    import hashlib
    key = hashlib.sha1(
        np.ascontiguousarray(np.asarray(edge_index, dtype=np.int64))
    ).hexdigest()
    if key not in _CACHE:
        plan, per_core = plan_and_inputs(edge_index, edge_weight)
        nc = build(plan)
        _CACHE[key] = (plan, per_core, nc)
    plan, per_core, nc = _CACHE[key]

    in_maps = make_in_maps(x, W, a_src, a_dst, ep_w, ep_b, bias, per_core)
    res = run_bass_kernel_spmd(nc, in_maps, core_ids=list(range(NCORES)),
                               trace=False)
    out_full = np.empty((NPAD, OUT_F), dtype=np.float32)
    for c in range(NCORES):
        out_full[c * NPC: (c + 1) * NPC] = res.results[c]["out"]
    # device columns are d-major (d*8+h); back to h*16+d
    out_full = out_full.reshape(-1, HD, H).transpose(0, 2, 1).reshape(
        -1, OUT_F)
    return out_full[:N]


# revision 9
# speedup vs baseline: 1.0838x; 1.0757x over previous
"""EnhancedCorrelationGNN Trainium2 kernel (8 NeuronCores, SPMD).

Strategy: destination-sorted edge processing with node-range output sharding,
fully collective-free.
 - Host (free): counting-sort edges by dst, partition nodes into 8 ranges of
   6272 (49 blocks x 128 nodes per core). Per core the node table is ROTATED
   so its own slice comes first; per block, edges are split by rotated src
   half (dma_gather int16 index limit) and padded to 128-edge tiles with
   cross-core-uniform tile counts (one SPMD program).
 - Phase 1 (device): EVERY core computes the FULL node table from the
   replicated x input: h = x @ W plus both attention projections in ONE bf16
   matmul per 128-node tile (rhs = [W@a_dst | W | W@a_src] prepped on host),
   writes bf16 [h|as] rows (512B) to local DRAM. attn_d for the core's own
   49 blocks stays in SBUF. No AllGather.
 - Phase 2 (device): per 32-tile chunk, one dma_gather of bf16 [h|as] rows
   by src (512B/edge); attn_d is expanded per-edge by a TensorE matmul with
   a host-shipped fp8 one-hot (node x edge) instead of a second gather.
   Scores: DVE adds + ACT Lrelu/Exp; messages bf16; one-hot segment matrix
   via is_equal(dstl, iota) in bf16; per-tile bf16 TensorE matmul
   scatter-accumulates [msgs | p] into the block PSUM; per block normalize
   by 1/(sum p + 1e-10), add bias, DMA out.
"""
import sys

if "/opt/trn_rl_repo" not in sys.path:
    sys.path.insert(0, "/opt/trn_rl_repo")

import numpy as np
import ml_dtypes

import concourse.bass as bass
import concourse.bacc as bacc
import concourse.mybir as mybir
import concourse.tile as tile
from concourse.bass_utils import run_bass_kernel_spmd

# ---------------------------------------------------------------- constants
N = 50000
E = 800000
IN_F = 128
H = 8
HD = 16
OUT_F = H * HD          # 128
ALPHA = 0.2
EPS = 1e-10

NCORES = 8
P = 128
NPC = 6272              # nodes per core = 49 * 128; 8*6272 = 50176 >= N
NPAD = NCORES * NPC     # 50176
NBLK = NPC // P         # 49
HALF = NPAD // 2        # 25088 rotated-table rows per gather stream

ROW = 256               # table row elems (bf16): h(128) | as(8) | pad -> 512B
AS_OFF = 128            # attn_s offset within row
CHUNK_TILES = 16        # tiles per gather/DVE chunk
IDX_COLS = CHUNK_TILES * P // 16   # wrapped int16 idx columns per chunk
PAD_DSTL = 300.0        # one-hot miss sentinel (exact in bf16)
XBLK = 28               # phase-1 blocks per xT chunk; 392 = 14 * 28
NXCH = (NPAD // P) // XBLK         # 14 phase-1 chunks (7 per half)

FP = mybir.dt.float32
BF = mybir.dt.bfloat16
F8 = mybir.dt.float8e4
NP_BF = ml_dtypes.bfloat16
NP_F8 = ml_dtypes.float8_e4m3

USE_FP8_S2 = True       # one-hot S2 dtype (fp8 halves its DMA vs bf16)


# ---------------------------------------------------------------- planning
def _cdiv(a, b):
    return -(-a // b)


def _wrap_idx(idx_flat: np.ndarray) -> np.ndarray:
    """[n] -> [128, IDX_COLS] int16: idx j at [j%16, j//16], replicated x8."""
    n = idx_flat.shape[0]
    assert n % 16 == 0
    w = idx_flat.reshape(n // 16, 16).T.astype(np.int16)      # [16, n/16]
    w = np.tile(w, (8, 1))                                    # [128, n/16]
    out = np.zeros((P, IDX_COLS), dtype=np.int16)
    out[:, : w.shape[1]] = w
    return out


def plan_and_inputs(edge_index, edge_weight):
    """Host-side edge partitioning. Returns (plan, per_core_arrays).

    plan (core-independent, defines the SPMD program):
      KA, KB: [NBLK] tiles per (block, half)
      chunks: list of dicts(stream, g0, nt) over stream-major tile ids
      block_tiles: per block, list of (chunk_id, slot) in matmul order
      tile_block: [T] block id of each global tile
      T, n_chunks
    per_core_arrays[c]:
      src_idx [n_chunks,128,IDX_COLS] i16 (stream-relative, rotated table)
      dst_idx [n_chunks,128,IDX_COLS] i16 (core-relative, for attn_d gather)
      ew8     [128, T*8] bf16 (edge weight replicated per head)
      st      [128, T*128] fp8 scatter one-hot: st[e, t*128+n] = (dstl==n)
    """
    src = np.asarray(edge_index[0], dtype=np.int64)
    dst = np.asarray(edge_index[1], dtype=np.int64)
    ew = np.asarray(edge_weight, dtype=np.float32)

    order = np.argsort(dst, kind="stable")
    src_s, dst_s, ew_s = src[order], dst[order], ew[order]

    # block boundaries over sorted dst
    blk_starts = np.searchsorted(dst_s, np.arange(0, NPAD + 1, P))
    # per (core, block, half) edge index lists (into the sorted arrays)
    cnt = np.zeros((NCORES, NBLK, 2), dtype=np.int64)
    lists = [[[None, None] for _ in range(NBLK)] for _ in range(NCORES)]
    rot_all = []
    for c in range(NCORES):
        rot = (src_s - c * NPC) % NPAD     # rotated src row per core
        rot_all.append(rot)
        for b in range(NBLK):
            g = c * NBLK + b
            lo, hi = blk_starts[g], blk_starts[g + 1]
            r = rot[lo:hi]
            mA = r < HALF
            idxs = np.arange(lo, hi)
            lists[c][b][0] = idxs[mA]
            lists[c][b][1] = idxs[~mA]
            cnt[c, b, 0] = mA.sum()
            cnt[c, b, 1] = (~mA).sum()

    KA = np.maximum(_cdiv(cnt[:, :, 0].max(axis=0), P), 1).astype(np.int64)
    KB = _cdiv(cnt[:, :, 1].max(axis=0), P).astype(np.int64)

    T_A = int(KA.sum())
    T_B = int(KB.sum())
    T = T_A + T_B
    cumKA = np.concatenate([[0], np.cumsum(KA)])
    cumKB = np.concatenate([[0], np.cumsum(KB)])

    # chunks: stream-major [0,T_A) then [T_A,T)
    chunks = []
    g = 0
    while g < T_A:
        nt = min(CHUNK_TILES, T_A - g)
        chunks.append(dict(stream=0, g0=g, nt=nt))
        g += nt
    while g < T:
        nt = min(CHUNK_TILES, T - g)
        chunks.append(dict(stream=1, g0=g, nt=nt))
        g += nt
    n_chunks = len(chunks)

    chunk_of = np.empty(T, dtype=np.int64)
    slot_of = np.empty(T, dtype=np.int64)
    for ci, ch in enumerate(chunks):
        chunk_of[ch["g0"]: ch["g0"] + ch["nt"]] = ci
        slot_of[ch["g0"]: ch["g0"] + ch["nt"]] = np.arange(ch["nt"])

    tile_block = np.empty(T, dtype=np.int64)
    block_tiles = []
    for b in range(NBLK):
        tl = []
        for k in range(KA[b]):
            gidx = cumKA[b] + k
            tile_block[gidx] = b
            tl.append((int(chunk_of[gidx]), int(slot_of[gidx])))
        for k in range(KB[b]):
            gidx = T_A + cumKB[b] + k
            tile_block[gidx] = b
            tl.append((int(chunk_of[gidx]), int(slot_of[gidx])))
        block_tiles.append(tl)

    plan = dict(KA=KA, KB=KB, T=T, T_A=T_A, chunks=chunks,
                block_tiles=block_tiles, tile_block=tile_block,
                n_chunks=n_chunks)

    # ---------------- per-core slot arrays
    s2_dt = NP_F8 if USE_FP8_S2 else NP_BF
    per_core = []
    for c in range(NCORES):
        src_rel = np.zeros((T, P), dtype=np.int16)
        dst_rel = np.zeros((T, P), dtype=np.int16)
        dstl = np.full((T, P), PAD_DSTL, dtype=np.float32)
        eww = np.zeros((T, P), dtype=np.float32)
        rot = rot_all[c]
        for b in range(NBLK):
            for half, K, cum, base in ((0, KA, cumKA, 0),
                                       (1, KB, cumKB, T_A)):
                idxs = lists[c][b][half]
                # src-sorted slots: consecutive gather rows are nearly
                # contiguous in DRAM (page locality)
                idxs = idxs[np.argsort(rot[idxs], kind="stable")]
                n = idxs.shape[0]
                g0 = base + cum[b]
                nslots = int(K[b]) * P
                s_loc = np.zeros(nslots, dtype=np.int64)
                d_loc = np.zeros(nslots, dtype=np.int64)
                dl = np.full(nslots, PAD_DSTL, dtype=np.float32)
                w = np.zeros(nslots, dtype=np.float32)
                if n:
                    s_loc[:n] = rot[idxs] - (HALF if half else 0)
                    d_loc[:n] = dst_s[idxs] - c * NPC
                    dl[:n] = (dst_s[idxs] - (c * NPC + b * P)).astype(
                        np.float32)
                    w[:n] = ew_s[idxs]
                src_rel[g0: g0 + int(K[b])] = s_loc.reshape(
                    int(K[b]), P).astype(np.int16)
                dst_rel[g0: g0 + int(K[b])] = d_loc.reshape(
                    int(K[b]), P).astype(np.int16)
                dstl[g0: g0 + int(K[b])] = dl.reshape(int(K[b]), P)
                eww[g0: g0 + int(K[b])] = w.reshape(int(K[b]), P)

        src_idx = np.zeros((n_chunks, P, IDX_COLS), dtype=np.int16)
        dst_idx = np.zeros((n_chunks, P, IDX_COLS), dtype=np.int16)
        for ci, ch in enumerate(chunks):
            g0, nt = ch["g0"], ch["nt"]
            src_idx[ci] = _wrap_idx(src_rel[g0: g0 + nt].reshape(nt * P))
            dst_idx[ci] = _wrap_idx(dst_rel[g0: g0 + nt].reshape(nt * P))

        # scatter one-hot S [edge-partition, T*128 node cols]:
        # st[e, t*128+n] = (dstl[t, e] == n)
        st = (dstl.T[:, :, None]
              == np.arange(P, dtype=np.float32)[None, None, :]).astype(s2_dt)
        # ew replicated per head: [128, T*8]
        ew8 = np.repeat(eww.T[:, :, None], H, axis=2).reshape(P, T * H)

        per_core.append(dict(
            src_idx=src_idx, dst_idx=dst_idx,
            ew8=np.ascontiguousarray(ew8).astype(NP_BF),       # [128, T*8]
            st=np.ascontiguousarray(st.reshape(P, T * P)),     # [128, T*128]
        ))

    return plan, per_core


# build stages for HW bisection: 1=phase1+table only, 2=+gathers+s2,
# 3=+score/rhs pipeline, 4=full (default)
BUILD_STAGE = 4
# repeat whole kernel body inside one NEFF (for timing by differencing)
REPS = 1


# ---------------------------------------------------------------- builder
def build(plan):
    n_chunks = plan["n_chunks"]
    chunks = plan["chunks"]
    T = plan["T"]
    tile_block = plan["tile_block"]

    nc = bacc.Bacc("TRN2", target_bir_lowering=False, debug=False,
                   num_devices=NCORES, num_swdge_queues=4)
    qctr = [0]
    S2DT = F8 if USE_FP8_S2 else BF

    # inputs
    xt_in = nc.dram_tensor("xt_in", [P, NPAD], BF, kind="ExternalInput")
    rhsw_in = nc.dram_tensor("rhsw_in", [P, IN_F + 2 * H], BF,
                             kind="ExternalInput")
    epw_in = nc.dram_tensor("epw_in", [P, H], BF, kind="ExternalInput")
    epb_in = nc.dram_tensor("epb_in", [P, H], BF, kind="ExternalInput")
    bias_in = nc.dram_tensor("bias_in", [P, OUT_F], FP, kind="ExternalInput")
    ew8_in = nc.dram_tensor("ew8_in", [P, T * H], BF, kind="ExternalInput")
    sidx_in = nc.dram_tensor("sidx_in", [n_chunks, P, IDX_COLS],
                             mybir.dt.int16, kind="ExternalInput")
    didx_in = nc.dram_tensor("didx_in", [n_chunks, P, IDX_COLS],
                             mybir.dt.int16, kind="ExternalInput")
    st_in = nc.dram_tensor("st_in", [P, T * P], S2DT, kind="ExternalInput")
    out = nc.dram_tensor("out", [NPC, OUT_F], FP, kind="ExternalOutput")

    with tile.TileContext(nc) as tc:
        for _rep in range(REPS):
            with tc.tile_pool(name="dram", bufs=1, space="DRAM") as dram, \
                 tc.tile_pool(name="statics", bufs=1) as statics:

                hs_A = dram.tile([HALF, ROW], BF)
                hs_B = dram.tile([HALF, ROW], BF)
                ad_pad = dram.tile([NPC, P], BF)

                # ---------------- statics
                epw_sb = statics.tile([P, H], BF)
                nc.sync.dma_start(epw_sb[:], epw_in[:])
                epb_sb = statics.tile([P, H], BF)
                nc.sync.dma_start(epb_sb[:], epb_in[:])
                bias_sb = statics.tile([P, OUT_F], FP)
                nc.sync.dma_start(bias_sb[:], bias_in[:])
                rhsw_sb = statics.tile([P, IN_F + 2 * H], BF)
                nc.sync.dma_start(rhsw_sb[:], rhsw_in[:])
                # attn_d for this core's own 49 blocks (rotated blocks 0..48)
                ad_blk = statics.tile([P, NBLK * H], BF)

                # ---------------- phase 1: full node table, local
                with tc.tile_pool(name="p1x", bufs=2) as p1x, \
                     tc.tile_pool(name="p1s", bufs=3) as p1s, \
                     tc.tile_pool(name="p1ps", bufs=4, space="PSUM") as p1ps:
                    for ch in range(NXCH):
                        xc = p1x.tile([P, XBLK * P], BF, tag="xc")
                        nc.sync.dma_start(
                            xc[:], xt_in[:, ch * XBLK * P:
                                         (ch + 1) * XBLK * P])
                        stage = p1s.tile([P, XBLK * ROW], BF, tag="stage")
                        CW = IN_F + 2 * H
                        for k in range(0, XBLK, 2):
                            g = ch * XBLK + k
                            hpsum = p1ps.tile([P, 2 * CW], FP, space="PSUM")
                            for j in range(2):
                                nc.tensor.matmul(
                                    out=hpsum[:, j * CW: (j + 1) * CW],
                                    lhsT=xc[:, (k + j) * P: (k + j + 1) * P],
                                    rhs=rhsw_sb[:], start=True, stop=True)
                            # [ad | h | as] -> row [h|as], own-slice ad kept
                            nc.scalar.activation(
                                stage[:, k * ROW: (k + 2) * ROW].rearrange(
                                    "p (t f) -> p t f", f=ROW)[:, :, 0:IN_F + H],
                                hpsum[:].rearrange("p (t f) -> p t f",
                                                   f=CW)[:, :, H: 2 * H + IN_F],
                                mybir.ActivationFunctionType.Copy)
                            nown = max(0, min(NBLK - g, 2))
                            if nown:
                                nc.scalar.activation(
                                    ad_blk[:, g * H: (g + nown) * H],
                                    hpsum[:].rearrange(
                                        "p (t f) -> p t f",
                                        f=CW)[:, :nown, 0:H],
                                    mybir.ActivationFunctionType.Copy)
                        half_t = hs_A if ch < NXCH // 2 else hs_B
                        r0 = (ch % (NXCH // 2)) * XBLK * P
                        nc.sync.dma_start(
                            half_t[r0: r0 + XBLK * P, :].rearrange(
                                "(t p) f -> p t f", p=P),
                            stage[:].rearrange("p (t f) -> p t f", f=ROW))

                # own-slice attn_d rows -> DRAM table (gathered per edge)
                nc.sync.dma_start(
                    ad_pad[:].rearrange("(t p) f -> p t f", p=P)[:, :, 0:H],
                    ad_blk[:].rearrange("p (t h) -> p t h", h=H))

                # ---------------- phase 2
                with tc.tile_pool(name="meta", bufs=1) as meta, \
                     tc.tile_pool(name="gp", bufs=3) as gp, \
                     tc.tile_pool(name="s2p", bufs=3) as s2p, \
                     tc.tile_pool(name="sp", bufs=3) as sp, \
                     tc.tile_pool(name="rp", bufs=3) as rp, \
                     tc.tile_pool(name="ep", bufs=3) as ep, \
                     tc.tile_pool(name="op", bufs=3) as opool, \
                         tc.tile_pool(name="bps", bufs=4, space="PSUM") as bps:

                    ew8_sb = meta.tile([P, T * H], BF)
                    nc.sync.dma_start(ew8_sb[:], ew8_in[:])
                    sidx_all = meta.tile([P, n_chunks, IDX_COLS],
                                         mybir.dt.int16)
                    nc.sync.dma_start(
                        sidx_all[:],
                        sidx_in[:].rearrange("c p i -> p c i"))
                    didx_all = meta.tile([P, n_chunks, IDX_COLS],
                                         mybir.dt.int16)
                    nc.sync.dma_start(
                        didx_all[:],
                        didx_in[:].rearrange("c p i -> p c i"))

                    # e4_all = ew*epw + epb for every slot, upfront
                    e4_all = meta.tile([P, T * H], BF)
                    e4v_all = e4_all[:].rearrange("p (t h) -> p t h", h=H)
                    nc.vector.tensor_tensor(
                        out=e4v_all,
                        in0=ew8_sb[:].rearrange("p (t h) -> p t h", h=H),
                        in1=epw_sb[:].unsqueeze(1).broadcast_to([P, T, H]),
                        op=mybir.AluOpType.mult)
                    nc.vector.tensor_tensor(
                        out=e4v_all, in0=e4v_all,
                        in1=epb_sb[:].unsqueeze(1).broadcast_to([P, T, H]),
                        op=mybir.AluOpType.add)

                    chunk_tiles = {}

                    def emit_chunk(ci):
                        ch = chunks[ci]
                        g0, nt = ch["g0"], ch["nt"]
                        nidx = nt * P
                        n16 = nidx // 16
                        if BUILD_STAGE == 1:
                            return

                        gbuf = gp.tile([P, CHUNK_TILES, ROW], BF, tag="gbuf")
                        half_ap = (hs_A[:] if ch["stream"] == 0 else hs_B[:])
                        nc.gpsimd.dma_gather(
                            out_ap=gbuf[:, :nt, :], in_ap=half_ap,
                            idxs_ap=sidx_all[:, ci, :n16],
                            num_idxs=nidx, num_idxs_reg=nidx, elem_size=ROW,
                            single_packet=False, queue_num=qctr[0] % 4)
                        qctr[0] += 1

                        adbuf = s2p.tile([P, CHUNK_TILES, P], BF,
                                         tag="adbuf")
                        nc.gpsimd.dma_gather(
                            out_ap=adbuf[:, :nt, :], in_ap=ad_pad[:],
                            idxs_ap=didx_all[:, ci, :n16],
                            num_idxs=nidx, num_idxs_reg=nidx, elem_size=P,
                            single_packet=False, queue_num=qctr[0] % 4)
                        qctr[0] += 1
                        if BUILD_STAGE == 2:
                            chunk_tiles[ci] = (gbuf, adbuf)
                            return

                        # one-hot S [P, nt*128] shipped fp8 from host
                        s_t = sp.tile([P, CHUNK_TILES * P], S2DT, tag="s_t")
                        nc.sync.dma_start(
                            s_t[:, : nt * P],
                            st_in[:, g0 * P: (g0 + nt) * P])

                        # scores: e0 = as + ad ; e2 = lrelu(e0) ;
                        # e5 = e2 + e4 ; p = exp(e5)
                        rhs = rp.tile([P, CHUNK_TILES * (OUT_F + H)], BF,
                                      tag="rhs")
                        rhs_v = rhs[:].rearrange("p (t f) -> p t f",
                                                 f=OUT_F + H)
                        if BUILD_STAGE != 3.5:
                            emit_scores(ci, g0, nt, gbuf, adbuf, rhs_v)
                        if BUILD_STAGE == 3.25:
                            chunk_tiles[ci] = (s_t, rhs)
                            return
                        # msgs = h * p; h is d-major so innermost dim is
                        # the head axis (stride 1) for every operand -> 2x
                        nc.vector.tensor_tensor(
                            out=rhs_v[:, :nt, 0:OUT_F].rearrange(
                                "p t (d h) -> p t d h", h=H),
                            in0=gbuf[:, :nt, 0:IN_F].rearrange(
                                "p t (d h) -> p t d h", h=H),
                            in1=rhs_v[:, :nt, OUT_F: OUT_F + H].unsqueeze(2)
                                .broadcast_to([P, nt, HD, H]),
                            op=mybir.AluOpType.mult)
                        chunk_tiles[ci] = (s_t, rhs)

                    def emit_scores(ci, g0, nt, gbuf, adbuf, rhs_v):
                        e0 = ep.tile([P, CHUNK_TILES * H], BF, tag="e0")
                        e0v = e0[:].rearrange("p (t h) -> p t h",
                                              h=H)[:, :nt, :]
                        nc.vector.tensor_tensor(
                            out=e0v, in0=gbuf[:, :nt, AS_OFF: AS_OFF + H],
                            in1=adbuf[:, :nt, 0:H],
                            op=mybir.AluOpType.add)
                        e1 = ep.tile([P, CHUNK_TILES * H], BF, tag="e1")
                        e1v = e1[:].rearrange("p (t h) -> p t h",
                                              h=H)[:, :nt, :]
                        nc.vector.tensor_scalar_mul(out=e1v, in0=e0v,
                                                    scalar1=ALPHA)
                        e2 = ep.tile([P, CHUNK_TILES * H], BF, tag="e2")
                        e2v = e2[:].rearrange("p (t h) -> p t h",
                                              h=H)[:, :nt, :]
                        nc.vector.tensor_tensor(out=e2v, in0=e0v, in1=e1v,
                                                op=mybir.AluOpType.max)
                        e5 = ep.tile([P, CHUNK_TILES * H], BF, tag="e5")
                        e5v = e5[:].rearrange("p (t h) -> p t h",
                                              h=H)[:, :nt, :]
                        nc.vector.tensor_tensor(
                            out=e5v, in0=e2v,
                            in1=e4v_all[:, g0: g0 + nt, :],
                            op=mybir.AluOpType.add)

                        nc.scalar.activation(
                            rhs_v[:, :nt, OUT_F: OUT_F + H], e5v,
                            mybir.ActivationFunctionType.Exp)

                    if BUILD_STAGE < 4:
                        for ci in range(n_chunks):
                            emit_chunk(ci)
                        dump = opool.tile([P, OUT_F], FP, tag="dump")
                        if BUILD_STAGE == 1:
                            nc.vector.memset(dump[:], 0.0)
                        elif BUILD_STAGE == 2:
                            g0buf = chunk_tiles[0][0]
                            nc.vector.tensor_copy(dump[:],
                                                  g0buf[:, 0, 0:OUT_F])
                        else:
                            r0 = chunk_tiles[0][1]
                            nc.vector.tensor_copy(dump[:], r0[:, 0:OUT_F])
                        for b in range(NBLK):
                            nc.sync.dma_start(out[b * P: (b + 1) * P, :],
                                              dump[:])

                    for b in range(NBLK if BUILD_STAGE >= 4 else 0):
                        tl = plan["block_tiles"][b]
                        for (ci, slot) in tl:
                            if ci not in chunk_tiles:
                                emit_chunk(ci)
                        psum_b = bps.tile([P, OUT_F + H], FP, space="PSUM",
                                          tag="psum_b")
                        for i, (ci, slot) in enumerate(tl):
                            s_t, rhs = chunk_tiles[ci]
                            nc.tensor.matmul(
                                out=psum_b[:],
                                lhsT=s_t[:, slot * P: (slot + 1) * P],
                                rhs=rhs[:, slot * (OUT_F + H):
                                        (slot + 1) * (OUT_F + H)],
                                start=(i == 0), stop=(i == len(tl) - 1))
                        # normalize + bias
                        s_eps = opool.tile([P, H], FP, tag="s_eps")
                        nc.vector.tensor_scalar_add(
                            out=s_eps[:], in0=psum_b[:, OUT_F: OUT_F + H],
                            scalar1=EPS)
                        rcp = opool.tile([P, H], FP, tag="rcp")
                        nc.vector.reciprocal(rcp[:], s_eps[:])
                        ob1 = opool.tile([P, OUT_F], FP, tag="ob1")
                        nc.vector.tensor_tensor(
                            out=ob1[:].rearrange("p (d h) -> p d h", h=H),
                            in0=psum_b[:, 0:OUT_F].rearrange(
                                "p (d h) -> p d h", h=H),
                            in1=rcp[:].unsqueeze(1).broadcast_to([P, HD, H]),
                            op=mybir.AluOpType.mult)
                        ob2 = opool.tile([P, OUT_F], FP, tag="ob2")
                        nc.vector.tensor_tensor(out=ob2[:], in0=ob1[:],
                                                in1=bias_sb[:],
                                                op=mybir.AluOpType.add)
                        nc.sync.dma_start(out[b * P: (b + 1) * P, :], ob2[:])

    nc.compile()
    # SWDGE constraint: a DMA semaphore may only be updated from one queue.
    # Tile assigns DMASW lanes post-scheduling, so align queue_num to lane.
    for f in nc.m.functions:
        for bb in f.blocks:
            for ins in bb.instructions:
                if type(ins).__name__ == "InstDMAGatherAnt":
                    si = ins.sync_info
                    lane = None
                    for u in si.on_update:
                        nm = u.ant_name or ""
                        if nm.startswith("DMASW"):
                            lane = int(nm[5:].split("_")[0])
                            break
                    assert lane is not None, "gather without DMASW sem"
                    ins.queue_num = lane % 4
    return nc


# ---------------------------------------------------------------- host API
def make_in_maps(x, W, a_src, a_dst, ep_w, ep_b, bias, per_core):
    x = np.asarray(x, dtype=np.float32)
    W = np.asarray(W, dtype=np.float32)
    a_src = np.asarray(a_src, dtype=np.float32)
    a_dst = np.asarray(a_dst, dtype=np.float32)
    ep_w = np.asarray(ep_w, dtype=np.float32)
    ep_b = np.asarray(ep_b, dtype=np.float32)
    bias = np.asarray(bias, dtype=np.float32)

    x_pad = np.zeros((NPAD, IN_F), dtype=np.float32)
    x_pad[:N] = x
    # rhs_w = [W@a_dst | W (d-major cols) | W@a_src]  [IN, 144]
    w_flat = W.transpose(1, 2, 0).reshape(IN_F, HD * H)       # [IN, d*8+h]
    wad = np.einsum('hid,hd->ih', W, a_dst)                   # [IN, H]
    was = np.einsum('hid,hd->ih', W, a_src)                   # [IN, H]
    rhs_w = np.concatenate([wad, w_flat, was], axis=1).astype(NP_BF)
    bias_dm = bias.reshape(H, HD).T.reshape(OUT_F)            # d-major

    rep = lambda v, dt: np.ascontiguousarray(
        np.broadcast_to(v[None, :], (P, v.shape[0]))).astype(dt)

    maps = []
    for c in range(NCORES):
        pc = per_core[c]
        x_rot = np.roll(x_pad, -c * NPC, axis=0)
        xt = np.ascontiguousarray(x_rot.T).astype(NP_BF)
        maps.append({
            "xt_in": xt,
            "rhsw_in": np.ascontiguousarray(rhs_w),
            "epw_in": rep(ep_w, NP_BF),
            "epb_in": rep(ep_b, NP_BF),
            "bias_in": rep(bias_dm, np.float32),
            "ew8_in": pc["ew8"],
            "sidx_in": pc["src_idx"],
            "didx_in": pc["dst_idx"],
            "st_in": pc["st"],
        })
    return maps


_CACHE = {}


def kernel(x, edge_index, edge_weight, W, a_src, a_dst, ep_w, ep_b, bias):
    import hashlib
    key = hashlib.sha1(
        np.ascontiguousarray(np.asarray(edge_index, dtype=np.int64))
    ).hexdigest()
    if key not in _CACHE:
        plan, per_core = plan_and_inputs(edge_index, edge_weight)
        nc = build(plan)
        _CACHE[key] = (plan, per_core, nc)
    plan, per_core, nc = _CACHE[key]

    in_maps = make_in_maps(x, W, a_src, a_dst, ep_w, ep_b, bias, per_core)
    res = run_bass_kernel_spmd(nc, in_maps, core_ids=list(range(NCORES)),
                               trace=False)
    out_full = np.empty((NPAD, OUT_F), dtype=np.float32)
    for c in range(NCORES):
        out_full[c * NPC: (c + 1) * NPC] = res.results[c]["out"]
    # device columns are d-major (d*8+h); back to h*16+d
    out_full = out_full.reshape(-1, HD, H).transpose(0, 2, 1).reshape(
        -1, OUT_F)
    return out_full[:N]
